# revision 46
# baseline (speedup 1.0000x reference)
"""Trainium2 Bass kernel for nn_CompressedCausalAttention.

Sharding: 8 cores = 2 batches x 4 head-groups (2 heads each).
Host precomputes xpe = (x+pe)^T in bf16 (kills the f32 x/pe DMA + on-chip adds).
Per-core dataflow, one fused loop per 512-wide s-window w:
  QKV(w):  q/k psum matmuls + bias-cast on DVE -> qT,kT bf16
           v psum matmuls (packed 128-ch) -> vsb bf16 (ones col 0 = denom)
  ATTN(w): per t-chunk j: scores (t-part, s-free) into PSUM, staircase mask
           via PE tri-matmul, Exp on ACT -> pb bf16, AV matmul accumulates
           (row 0 of avs = softmax denominator via the ones column)
  POST(w): reciprocal_approx_fast(denom row, PSUM base-0) on DVE -> GPSIMD
           partition_broadcast -> per-head normalize muls on DVE
  OUT(w):  4 out-projection matmuls, PSUM->SBUF f32 copies on DVE, DMA out
The attention stream is software-pipelined (each chunk's AV emitted after the
next chunk's score matmuls) and QKV(w+1)/outproj matmuls are interleaved into
it as PE filler, so the in-order PE never stalls on the ACT exp and its DVFS
p-state ramps to full clock.
Host: sums the 4 per-batch f32 partials, adds bc (+ v-bias folded through Wc).
"""

import numpy as np
import ml_dtypes

S, B, C, H = 2048, 2, 512, 8
CC = C // H            # 64
HPC = 2                # heads per core
CPC = HPC * CC         # 128 channels per core
NCORE = 8
SW = 512               # s window (free dim of score tiles)
TCH = 128              # t chunk (partition dim of score tiles)
TEMP = 1.0 / 8.0       # 1/sqrt(CC)
BIGNEG = -30000.0

_CACHE = {}


def _build_bass():
    import concourse.bass as bass
    import concourse.mybir as mybir
    import concourse.tile as tile
    from concourse import bacc

    f32 = mybir.dt.float32
    bf16 = mybir.dt.bfloat16

    nc = bacc.Bacc("TRN2", target_bir_lowering=False)
    xpet = nc.declare_dram_parameter("xpet", [4, 4, 128, SW], bf16, isOutput=False)
    w3t = nc.declare_dram_parameter("w3t", [4, 128, 384], bf16, isOutput=False)
    b3 = nc.declare_dram_parameter("b3", [128, 2], f32, isOutput=False)
    wct = nc.declare_dram_parameter("wct", [128, C], bf16, isOutput=False)
    tri = nc.declare_dram_parameter("tri", [128, 128], bf16, isOutput=False)
    outp = nc.declare_dram_parameter("outp", [4, 4, 128, SW], f32, isOutput=True)

    Exp = mybir.ActivationFunctionType.Exp

    with tile.TileContext(nc) as tc:
        with (
            tc.tile_pool(name="singles", bufs=1) as singles,
            tc.tile_pool(name="pbp", bufs=3) as pbp,
            tc.tile_pool(name="atp", bufs=4) as atp,
            tc.tile_pool(name="rcp", bufs=2) as rcp,
            tc.tile_pool(name="osp", bufs=4) as osp,
            tc.tile_pool(name="psS", bufs=2, space="PSUM") as psS,
            tc.tile_pool(name="psAV", bufs=1, space="PSUM") as psAV,
            tc.tile_pool(name="psQ", bufs=2, space="PSUM") as psQ,
        ):
            # ---- constants ----
            w3tk = [singles.tile([128, 384], bf16, tag=f"w3t{k}", name=f"w3t{k}")
                    for k in range(4)]
            for k in range(4):
                nc.sync.dma_start(out=w3tk[k][:, :], in_=w3t[k])
            b3_sb = singles.tile([128, 2], f32, tag="b3")
            nc.sync.dma_start(out=b3_sb, in_=b3[:, :])
            wct_sb = singles.tile([128, C], bf16, tag="wct")
            nc.sync.dma_start(out=wct_sb, in_=wct[:, :])
            # 0/1 upper-tri (col >= row) mask pair, applied to pb after exp
            tri_sb = singles.tile([128, 2, 128], bf16, tag="tri")
            for h in range(HPC):
                nc.sync.dma_start(out=tri_sb[:, h, :], in_=tri[:, :])

            # xpe^T, prefetched up front: one contiguous single-descriptor DMA
            # per (c-chunk, window) tile to keep descriptor overhead tiny
            xpT = [[singles.tile([128, SW], bf16, tag=f"xpT{k}_{w}",
                                 name=f"xpT{k}_{w}") for w in range(4)]
                   for k in range(4)]
            for w in range(4):
                for k in range(4):
                    nc.sync.dma_start(out=xpT[k][w][:, :], in_=xpet[k, w])

            qT = singles.tile([128, S], bf16, tag="qT")
            kT = singles.tile([128, S], bf16, tag="kT")
            # v seq-major; col 0 = ones so row 0 of avs is the softmax
            # denominator (partition 0 — reciprocal_approx_fast needs base-0
            # PSUM reads); v channels at cols 64..127 (32-aligned partition
            # base for the normalize multiply); cols 1..63 zero
            vsb = singles.tile([128, 16, 2, 128], bf16, tag="vsb")
            nc.vector.memset(vsb[:, :, :, 1:CC], 0.0)
            nc.vector.memset(vsb[:, :, :, 0:1], 1.0)

            def qkv_fillers(w):
                """PE-matmul closures for window w's QKV, emitted one at a
                time between attention steps of window w-1."""
                sl = slice(w * SW, (w + 1) * SW)
                fill = []
                state = {}

                def qk_mm(blk, k, sl=sl):
                    if k == 0:
                        state[blk] = psQ.tile([128, SW], f32, tag="q",
                                              name=f"qk{blk}_{w}")
                    ps = state[blk]
                    nc.tensor.matmul(
                        ps, lhsT=w3tk[k][:, blk * 128:(blk + 1) * 128],
                        rhs=xpT[k][w][:, :], start=(k == 0), stop=(k == 3),
                        skip_group_check=True,
                    )
                    if k == 3:
                        nc.vector.tensor_scalar_add(
                            (qT if blk == 0 else kT)[:, sl], ps,
                            b3_sb[:, blk:blk + 1])

                def v_mm(tc_, k, w=w):
                    if tc_ == 0 and k == 0:
                        state["v"] = psQ.tile([128, 4, 2, CC], f32, tag="q",
                                              name=f"vps_{w}")
                    vps = state["v"]
                    nc.tensor.matmul(
                        vps[:, tc_],
                        lhsT=xpT[k][w][:, tc_ * 128:(tc_ + 1) * 128],
                        rhs=w3tk[k][:, 256:384], start=(k == 0), stop=(k == 3),
                        skip_group_check=True,
                    )
                    if tc_ == 3 and k == 3:
                        nc.vector.tensor_copy(
                            out=vsb[:, 4 * w:4 * w + 4, :, CC:], in_=vps)

                for blk in range(2):
                    for k in range(4):
                        fill.append(lambda blk=blk, k=k: qk_mm(blk, k))
                for tc_ in range(4):
                    for k in range(4):
                        fill.append(lambda tc_=tc_, k=k: v_mm(tc_, k))
                return fill

            def outproj_fillers(w, atn, copy_eng=None):
                fill = []

                def op_mm(d, w=w, atn=atn):
                    op = psQ.tile([128, SW], f32, tag="q", name=f"op{d}_{w}")
                    nc.tensor.matmul(
                        op, lhsT=wct_sb[:, d * 128:(d + 1) * 128], rhs=atn,
                        start=True, stop=True, skip_group_check=True,
                    )
                    ob = osp.tile([128, SW], f32, tag="ob", name=f"ob{d}_{w}")
                    if copy_eng is None:
                        nc.vector.tensor_copy(out=ob, in_=op)
                    else:
                        copy_eng.copy(out=ob, in_=op)
                    nc.sync.dma_start(out=outp[w, d], in_=ob)

                for d in range(4):
                    fill.append(lambda d=d: op_mm(d))
                return fill

            # window 0's QKV runs up front; later windows' QKV and the
            # out-projections ride the attention streams as filler
            for f in qkv_fillers(0):
                f()

            atns = []
            for w in range(4):
                fillers = []
                if w < 3:
                    fillers += qkv_fillers(w + 1)
                if w == 3:
                    for pw, patn in enumerate(atns):
                        fillers += outproj_fillers(pw, patn)

                # ---- attention for this window (software-pipelined) ----
                jmax = 4 * w + 3
                avs = [psAV.tile([128, SW], f32, tag=f"av{h}", name=f"av{h}_{w}")
                       for h in range(HPC)]
                nitems = jmax + 1
                pend = None
                fi = 0
                for j in range(nitems):
                    # D = first valid column of this (t-chunk, s-window) pair
                    D = max(0, 128 * j - 512 * w)
                    sc = psS.tile([128, 2, SW], f32, tag="sc")
                    for h in range(HPC):
                        nc.tensor.matmul(
                            sc[:, h, D:SW],
                            lhsT=kT[h * CC:(h + 1) * CC, j * TCH:(j + 1) * TCH],
                            rhs=qT[h * CC:(h + 1) * CC, w * SW + D:(w + 1) * SW],
                            start=True, stop=True,
                        )
                    pb = pbp.tile([128, 2, SW], bf16, tag="pb")
                    nc.scalar.activation(out=pb[:, :, D:SW], in_=sc[:, :, D:SW],
                                         func=Exp, scale=TEMP)
                    if j >= 4 * w:
                        # staircase: zero the strictly-future wedge of the
                        # diagonal 128-block (all-bf16 SBUF -> fast DVE mode)
                        nc.vector.tensor_tensor(
                            out=pb[:, :, D:D + 128], in0=pb[:, :, D:D + 128],
                            in1=tri_sb[:, :, :], op=mybir.AluOpType.mult,
                        )
                    # spread this window's filler matmuls across its steps
                    # (front-loaded into the first nitems-2 steps so the DVE
                    # queue is drained before the post-chain recips)
                    ntake = min(len(fillers),
                                (len(fillers) * (j + 1)) // max(1, nitems - 2))
                    while fi < ntake:
                        fillers[fi]()
                        fi += 1

                    def av_emit(j=j, D=D, pb=pb, avs=avs, jmax=jmax):
                        for h in range(HPC):
                            nc.tensor.matmul(
                                avs[h][:, D:SW], lhsT=vsb[:, j, h, :],
                                rhs=pb[:, h, D:SW],
                                start=(j == 0), stop=(j == jmax),
                                skip_group_check=True,
                            )
                    if pend is not None:
                        pend()
                    pend = av_emit
                pend()
                while fi < len(fillers):
                    fillers[fi]()
                    fi += 1

                # ---- normalize: 1/denom (row 0 of avs), broadcast, multiply ----
                # (both recips first so h1's DVE work hides under h0's GPSIMD
                # broadcast)
                atn = atp.tile([128, SW], bf16, tag="atn", name=f"atn{w}")
                rcs, brcs = [], []
                for h in range(HPC):
                    rc = rcp.tile([1, SW], f32, tag=f"rc{h}", name=f"rc{h}_{w}")
                    nc.vector.reciprocal_approx_fast(out=rc, in_=avs[h][0:1, :])
                    rcs.append(rc)
                for h in range(HPC):
                    brc = rcp.tile([128, SW], f32, tag=f"brc{h}", name=f"brc{h}_{w}")
                    nc.gpsimd.partition_broadcast(brc[:], rcs[h][:])
                    brcs.append(brc)
                for h in range(HPC):
                    nc.vector.tensor_tensor(
                        out=atn[h * CC:(h + 1) * CC, :],
                        in0=avs[h][CC:, :], in1=brcs[h][CC:, :],
                        op=mybir.AluOpType.mult,
                    )
                atns.append(atn)

            for f in outproj_fillers(3, atns[3], copy_eng=nc.scalar):
                f()

    nc.compile()
    return nc


def _get_nc():
    if "nc" not in _CACHE:
        _CACHE["nc"] = _build_bass()
    return _CACHE["nc"]


def _make_in_maps(x, pe, Wqkv, bqkv, Wc):
    bf = ml_dtypes.bfloat16
    tt = np.arange(128)[:, None]
    kk = np.arange(128)[None, :]
    # 0/1 mask: keep col >= row (past/diagonal), zero the strictly-future wedge
    tri = np.where(kk >= tt, np.float32(1.0), np.float32(0.0)).astype(bf)

    xpet_b = {}
    for b in range(B):
        t = (x[:, b, :] + pe[:, b, :]).T.astype(bf)   # (512c, 2048s)
        t = t.reshape(4, 128, 4, SW)                  # (k, 128, w, 512)
        xpet_b[b] = np.ascontiguousarray(t.transpose(0, 2, 1, 3))  # (k, w, ...)

    in_maps = []
    for core in range(NCORE):
        b, hg = core // 4, core % 4
        lo = hg * 128
        W3 = np.concatenate([Wqkv[lo:lo + 128], Wqkv[C + lo:C + lo + 128],
                             Wqkv[2 * C + lo:2 * C + lo + 128]])
        w3t = np.ascontiguousarray(W3.T).reshape(4, 128, 384).astype(bf)
        b3 = np.stack([bqkv[lo:lo + 128], bqkv[C + lo:C + lo + 128]], axis=1)
        b3 = np.ascontiguousarray(b3).astype(np.float32)
        wct = np.ascontiguousarray(Wc[:, lo:lo + 128].T).astype(bf)
        in_maps.append({
            "xpet": xpet_b[b], "w3t": w3t, "b3": b3,
            "wct": wct, "tri": tri,
        })
    return in_maps


def _numpy_fallback(x, pe, content_mask, Wqkv, bqkv, Wc, bc):
    xpe = (x + pe).astype(np.float32)
    qkv = xpe.reshape(-1, C) @ Wqkv.T + bqkv
    qkv = qkv.reshape(S, B, 3 * C)
    q, k, v = np.split(qkv, 3, axis=-1)
    q = q.reshape(S, B, H, CC)
    k = k.reshape(S, B, H, CC)
    v = v.reshape(S, B, H, CC)
    out = np.empty((S, B, C), np.float32)
    for b in range(B):
        for h in range(H):
            sc = (q[:, b, h] @ k[:, b, h].T) * np.float32(TEMP)
            sc = np.where(content_mask[:, :, b], -np.inf, sc)
            sc = sc - sc.max(axis=1, keepdims=True)
            p = np.exp(sc)
            p /= p.sum(axis=1, keepdims=True)
            out[:, b, h * CC:(h + 1) * CC] = p @ v[:, b, h]
    return (out.reshape(-1, C) @ Wc.T + bc).reshape(S, B, C).astype(np.float32)


def kernel(x, pe, content_mask, pad, Wqkv, bqkv, Wc, bc):
    x = np.asarray(x, dtype=np.float32)
    pe = np.asarray(pe, dtype=np.float32)
    content_mask = np.asarray(content_mask)
    Wqkv = np.asarray(Wqkv, dtype=np.float32)
    bqkv = np.asarray(bqkv, dtype=np.float32)
    Wc = np.asarray(Wc, dtype=np.float32)
    bc = np.asarray(bc, dtype=np.float32)

    idx = np.arange(S)
    causal = idx[None, :] > idx[:, None]
    if not np.array_equal(content_mask, np.broadcast_to(causal[:, :, None], (S, S, B))):
        return _numpy_fallback(x, pe, content_mask, Wqkv, bqkv, Wc, bc)

    from concourse.bass_utils import run_bass_kernel_spmd

    nc = _get_nc()
    in_maps = _make_in_maps(x, pe, Wqkv, bqkv, Wc)
    res = run_bass_kernel_spmd(nc, in_maps, core_ids=list(range(NCORE)))
    out = np.empty((S, B, C), np.float32)
    bc_eff = bc + Wc @ bqkv[2 * C:3 * C]   # v-bias folded through the output proj
    for b in range(B):
        acc = res.results[b * 4]["outp"].astype(np.float32).copy()
        for g in range(1, 4):
            acc += res.results[b * 4 + g]["outp"]
        # (w, d, 128, 512) -> (d*128, w*512) = (C, S)
        acc = acc.transpose(1, 2, 0, 3).reshape(C, S)
        out[:, b, :] = acc.T + bc_eff
    return out


# revision 47
# speedup vs baseline: 1.1991x; 1.1991x over previous
"""Trainium2 Bass kernel for nn_CompressedCausalAttention.

Sharding: 8 cores = 2 batches x 4 head-groups (2 heads each).
Host precomputes xpe = (x+pe)^T in bf16 (kills the f32 x/pe DMA + on-chip adds).
Per-core dataflow, one fused loop per 512-wide s-window w:
  QKV(w):  q/k psum matmuls + bias-cast on DVE -> qT,kT bf16
           v psum matmuls (packed 128-ch) -> vsb bf16 (ones col 0 = denom)
  ATTN(w): per t-chunk j: scores (t-part, s-free) into PSUM, staircase mask
           via PE tri-matmul, Exp on ACT -> pb bf16, AV matmul accumulates
           (row 0 of avs = softmax denominator via the ones column)
  POST(w): reciprocal_approx_fast(denom row, PSUM base-0) on DVE -> GPSIMD
           partition_broadcast -> per-head normalize muls on DVE
  OUT(w):  4 out-projection matmuls, PSUM->SBUF f32 copies on DVE, DMA out
The attention stream is software-pipelined (each chunk's AV emitted after the
next chunk's score matmuls) and QKV(w+1)/outproj matmuls are interleaved into
it as PE filler, so the in-order PE never stalls on the ACT exp and its DVFS
p-state ramps to full clock.
Host: sums the 4 per-batch f32 partials, adds bc (+ v-bias folded through Wc).
"""

import numpy as np
import ml_dtypes

S, B, C, H = 2048, 2, 512, 8
CC = C // H            # 64
HPC = 2                # heads per core
CPC = HPC * CC         # 128 channels per core
NCORE = 8
SW = 512               # s window (free dim of score tiles)
TCH = 128              # t chunk (partition dim of score tiles)
TEMP = 1.0 / 8.0       # 1/sqrt(CC)
BIGNEG = -30000.0

_CACHE = {}


def _build_bass():
    import concourse.bass as bass
    import concourse.mybir as mybir
    import concourse.tile as tile
    from concourse import bacc

    f32 = mybir.dt.float32
    bf16 = mybir.dt.bfloat16

    nc = bacc.Bacc("TRN2", target_bir_lowering=False)
    xpet = nc.declare_dram_parameter("xpet", [4, 4, 128, SW], bf16, isOutput=False)
    w3t = nc.declare_dram_parameter("w3t", [4, 128, 384], bf16, isOutput=False)
    b3 = nc.declare_dram_parameter("b3", [128, 2], f32, isOutput=False)
    wct = nc.declare_dram_parameter("wct", [128, C], bf16, isOutput=False)
    tri = nc.declare_dram_parameter("tri", [128, 128], bf16, isOutput=False)
    outp = nc.declare_dram_parameter("outp", [4, 4, 128, SW], f32, isOutput=True)

    Exp = mybir.ActivationFunctionType.Exp

    with tile.TileContext(nc) as tc:
        with (
            tc.tile_pool(name="singles", bufs=1) as singles,
            tc.tile_pool(name="pbp", bufs=3) as pbp,
            tc.tile_pool(name="atp", bufs=4) as atp,
            tc.tile_pool(name="rcp", bufs=2) as rcp,
            tc.tile_pool(name="osp", bufs=4) as osp,
            tc.tile_pool(name="psS", bufs=2, space="PSUM") as psS,
            tc.tile_pool(name="psAV", bufs=1, space="PSUM") as psAV,
            tc.tile_pool(name="psQ", bufs=2, space="PSUM") as psQ,
        ):
            # ---- constants ----
            w3tk = [singles.tile([128, 384], bf16, tag=f"w3t{k}", name=f"w3t{k}")
                    for k in range(4)]
            for k in range(4):
                nc.sync.dma_start(out=w3tk[k][:, :], in_=w3t[k])
            b3_sb = singles.tile([128, 2], f32, tag="b3")
            nc.sync.dma_start(out=b3_sb, in_=b3[:, :])
            wct_sb = singles.tile([128, C], bf16, tag="wct")
            nc.sync.dma_start(out=wct_sb, in_=wct[:, :])
            # 0/1 upper-tri (col >= row) mask pair, applied to pb after exp
            tri_sb = singles.tile([128, 2, 128], bf16, tag="tri")
            for h in range(HPC):
                nc.sync.dma_start(out=tri_sb[:, h, :], in_=tri[:, :])

            # xpe^T, prefetched up front: one contiguous single-descriptor DMA
            # per (c-chunk, window) tile to keep descriptor overhead tiny
            xpT = [[singles.tile([128, SW], bf16, tag=f"xpT{k}_{w}",
                                 name=f"xpT{k}_{w}") for w in range(4)]
                   for k in range(4)]
            for w in range(4):
                for k in range(4):
                    nc.sync.dma_start(out=xpT[k][w][:, :], in_=xpet[k, w])

            qT = singles.tile([128, S], bf16, tag="qT")
            kT = singles.tile([128, S], bf16, tag="kT")
            # v seq-major; col 0 = ones so row 0 of avs is the softmax
            # denominator (partition 0 — reciprocal_approx_fast needs base-0
            # PSUM reads); v channels at cols 64..127 (32-aligned partition
            # base for the normalize multiply); cols 1..63 zero
            vsb = singles.tile([128, 16, 2, 128], bf16, tag="vsb")
            nc.vector.memset(vsb[:, :, :, 1:CC], 0.0)
            nc.vector.memset(vsb[:, :, :, 0:1], 1.0)

            def qkv_fillers(w):
                """PE-matmul closures for window w's QKV, emitted one at a
                time between attention steps of window w-1."""
                sl = slice(w * SW, (w + 1) * SW)
                fill = []
                state = {}

                def qk_mm(blk, k, sl=sl):
                    if k == 0:
                        state[blk] = psQ.tile([128, SW], f32, tag="q",
                                              name=f"qk{blk}_{w}")
                    ps = state[blk]
                    nc.tensor.matmul(
                        ps, lhsT=w3tk[k][:, blk * 128:(blk + 1) * 128],
                        rhs=xpT[k][w][:, :], start=(k == 0), stop=(k == 3),
                        skip_group_check=True,
                    )
                    if k == 3:
                        nc.vector.tensor_scalar_add(
                            (qT if blk == 0 else kT)[:, sl], ps,
                            b3_sb[:, blk:blk + 1])

                def v_mm(tc_, k, w=w):
                    if tc_ == 0 and k == 0:
                        state["v"] = psQ.tile([128, 4, 2, CC], f32, tag="q",
                                              name=f"vps_{w}")
                    vps = state["v"]
                    nc.tensor.matmul(
                        vps[:, tc_],
                        lhsT=xpT[k][w][:, tc_ * 128:(tc_ + 1) * 128],
                        rhs=w3tk[k][:, 256:384], start=(k == 0), stop=(k == 3),
                        skip_group_check=True,
                    )
                    if tc_ == 3 and k == 3:
                        nc.vector.tensor_copy(
                            out=vsb[:, 4 * w:4 * w + 4, :, CC:], in_=vps)

                for blk in range(2):
                    for k in range(4):
                        fill.append(lambda blk=blk, k=k: qk_mm(blk, k))
                for tc_ in range(4):
                    for k in range(4):
                        fill.append(lambda tc_=tc_, k=k: v_mm(tc_, k))
                return fill

            def outproj_fillers(w, atn, copy_eng=None):
                fill = []

                def op_mm(d, w=w, atn=atn):
                    op = psQ.tile([128, SW], f32, tag="q", name=f"op{d}_{w}")
                    nc.tensor.matmul(
                        op, lhsT=wct_sb[:, d * 128:(d + 1) * 128], rhs=atn,
                        start=True, stop=True, skip_group_check=True,
                    )
                    ob = osp.tile([128, SW], f32, tag="ob", name=f"ob{d}_{w}")
                    if copy_eng is None:
                        nc.vector.tensor_copy(out=ob, in_=op)
                    else:
                        copy_eng.copy(out=ob, in_=op)
                    nc.sync.dma_start(out=outp[w, d], in_=ob)

                for d in range(4):
                    fill.append(lambda d=d: op_mm(d))
                return fill

            # window 0's QKV runs up front; later windows' QKV and the
            # out-projections ride the attention streams as filler
            for f in qkv_fillers(0):
                f()

            atns = []
            for w in range(4):
                fillers = []
                if w < 3:
                    fillers += qkv_fillers(w + 1)
                if w == 3:
                    for pw, patn in enumerate(atns):
                        fillers += outproj_fillers(pw, patn)

                # ---- attention for this window (software-pipelined) ----
                jmax = 4 * w + 3
                avs = [psAV.tile([128, SW], f32, tag=f"av{h}", name=f"av{h}_{w}")
                       for h in range(HPC)]
                nitems = jmax + 1
                pend = None
                fi = 0
                for j in range(nitems):
                    # D = first valid column of this (t-chunk, s-window) pair
                    D = max(0, 128 * j - 512 * w)
                    sc = psS.tile([128, 2, SW], f32, tag="sc")
                    for h in range(HPC):
                        nc.tensor.matmul(
                            sc[:, h, D:SW],
                            lhsT=kT[h * CC:(h + 1) * CC, j * TCH:(j + 1) * TCH],
                            rhs=qT[h * CC:(h + 1) * CC, w * SW + D:(w + 1) * SW],
                            start=True, stop=True,
                        )
                    pb = pbp.tile([128, 2, SW], bf16, tag="pb")
                    nc.scalar.activation(out=pb[:, :, D:SW], in_=sc[:, :, D:SW],
                                         func=Exp, scale=TEMP)
                    if j >= 4 * w:
                        # staircase: zero the strictly-future wedge of the
                        # diagonal 128-block (all-bf16 SBUF -> fast DVE mode)
                        nc.vector.tensor_tensor(
                            out=pb[:, :, D:D + 128], in0=pb[:, :, D:D + 128],
                            in1=tri_sb[:, :, :], op=mybir.AluOpType.mult,
                        )
                    # spread this window's filler matmuls across its steps
                    ntake = (len(fillers) * (j + 1)) // nitems
                    while fi < ntake:
                        fillers[fi]()
                        fi += 1

                    def av_emit(j=j, D=D, pb=pb, avs=avs, jmax=jmax):
                        for h in range(HPC):
                            nc.tensor.matmul(
                                avs[h][:, D:SW], lhsT=vsb[:, j, h, :],
                                rhs=pb[:, h, D:SW],
                                start=(j == 0), stop=(j == jmax),
                                skip_group_check=True,
                            )
                    if pend is not None:
                        pend()
                    pend = av_emit
                pend()
                while fi < len(fillers):
                    fillers[fi]()
                    fi += 1

                # ---- normalize: 1/denom (row 0 of avs), broadcast, multiply ----
                # (both recips first so h1's DVE work hides under h0's GPSIMD
                # broadcast)
                atn = atp.tile([128, SW], bf16, tag="atn", name=f"atn{w}")
                rcs, brcs = [], []
                for h in range(HPC):
                    rc = rcp.tile([1, SW], f32, tag=f"rc{h}", name=f"rc{h}_{w}")
                    nc.vector.reciprocal_approx_fast(out=rc, in_=avs[h][0:1, :])
                    rcs.append(rc)
                for h in range(HPC):
                    brc = rcp.tile([128, SW], f32, tag=f"brc{h}", name=f"brc{h}_{w}")
                    nc.gpsimd.partition_broadcast(brc[:], rcs[h][:])
                    brcs.append(brc)
                for h in range(HPC):
                    nc.vector.tensor_tensor(
                        out=atn[h * CC:(h + 1) * CC, :],
                        in0=avs[h][CC:, :], in1=brcs[h][CC:, :],
                        op=mybir.AluOpType.mult,
                    )
                atns.append(atn)

            for f in outproj_fillers(3, atns[3], copy_eng=nc.scalar):
                f()

    nc.compile()
    return nc


def _get_nc():
    if "nc" not in _CACHE:
        _CACHE["nc"] = _build_bass()
    return _CACHE["nc"]


def _make_in_maps(x, pe, Wqkv, bqkv, Wc):
    bf = ml_dtypes.bfloat16
    tt = np.arange(128)[:, None]
    kk = np.arange(128)[None, :]
    # 0/1 mask: keep col >= row (past/diagonal), zero the strictly-future wedge
    tri = np.where(kk >= tt, np.float32(1.0), np.float32(0.0)).astype(bf)

    xpet_b = {}
    for b in range(B):
        t = (x[:, b, :] + pe[:, b, :]).T.astype(bf)   # (512c, 2048s)
        t = t.reshape(4, 128, 4, SW)                  # (k, 128, w, 512)
        xpet_b[b] = np.ascontiguousarray(t.transpose(0, 2, 1, 3))  # (k, w, ...)

    in_maps = []
    for core in range(NCORE):
        b, hg = core // 4, core % 4
        lo = hg * 128
        W3 = np.concatenate([Wqkv[lo:lo + 128], Wqkv[C + lo:C + lo + 128],
                             Wqkv[2 * C + lo:2 * C + lo + 128]])
        w3t = np.ascontiguousarray(W3.T).reshape(4, 128, 384).astype(bf)
        b3 = np.stack([bqkv[lo:lo + 128], bqkv[C + lo:C + lo + 128]], axis=1)
        b3 = np.ascontiguousarray(b3).astype(np.float32)
        wct = np.ascontiguousarray(Wc[:, lo:lo + 128].T).astype(bf)
        in_maps.append({
            "xpet": xpet_b[b], "w3t": w3t, "b3": b3,
            "wct": wct, "tri": tri,
        })
    return in_maps


def _numpy_fallback(x, pe, content_mask, Wqkv, bqkv, Wc, bc):
    xpe = (x + pe).astype(np.float32)
    qkv = xpe.reshape(-1, C) @ Wqkv.T + bqkv
    qkv = qkv.reshape(S, B, 3 * C)
    q, k, v = np.split(qkv, 3, axis=-1)
    q = q.reshape(S, B, H, CC)
    k = k.reshape(S, B, H, CC)
    v = v.reshape(S, B, H, CC)
    out = np.empty((S, B, C), np.float32)
    for b in range(B):
        for h in range(H):
            sc = (q[:, b, h] @ k[:, b, h].T) * np.float32(TEMP)
            sc = np.where(content_mask[:, :, b], -np.inf, sc)
            sc = sc - sc.max(axis=1, keepdims=True)
            p = np.exp(sc)
            p /= p.sum(axis=1, keepdims=True)
            out[:, b, h * CC:(h + 1) * CC] = p @ v[:, b, h]
    return (out.reshape(-1, C) @ Wc.T + bc).reshape(S, B, C).astype(np.float32)


def kernel(x, pe, content_mask, pad, Wqkv, bqkv, Wc, bc):
    x = np.asarray(x, dtype=np.float32)
    pe = np.asarray(pe, dtype=np.float32)
    content_mask = np.asarray(content_mask)
    Wqkv = np.asarray(Wqkv, dtype=np.float32)
    bqkv = np.asarray(bqkv, dtype=np.float32)
    Wc = np.asarray(Wc, dtype=np.float32)
    bc = np.asarray(bc, dtype=np.float32)

    idx = np.arange(S)
    causal = idx[None, :] > idx[:, None]
    if not np.array_equal(content_mask, np.broadcast_to(causal[:, :, None], (S, S, B))):
        return _numpy_fallback(x, pe, content_mask, Wqkv, bqkv, Wc, bc)

    from concourse.bass_utils import run_bass_kernel_spmd

    nc = _get_nc()
    in_maps = _make_in_maps(x, pe, Wqkv, bqkv, Wc)
    res = run_bass_kernel_spmd(nc, in_maps, core_ids=list(range(NCORE)))
    out = np.empty((S, B, C), np.float32)
    bc_eff = bc + Wc @ bqkv[2 * C:3 * C]   # v-bias folded through the output proj
    for b in range(B):
        acc = res.results[b * 4]["outp"].astype(np.float32).copy()
        for g in range(1, 4):
            acc += res.results[b * 4 + g]["outp"]
        # (w, d, 128, 512) -> (d*128, w*512) = (C, S)
        acc = acc.transpose(1, 2, 0, 3).reshape(C, S)
        out[:, b, :] = acc.T + bc_eff
    return out
